# revision 1
# baseline (speedup 1.0000x reference)
"""TRN2 Bass kernel for nn_AttributeClassifierHeaders (dense per-head MLP).

Computes y[b, a] = sigmoid(gelu(x @ W1[a] + b1[a]) . W2[a] + b2[a]) for 40
heads, sharded 5 heads per NeuronCore across 8 cores (head-parallel: each
head's weights are independent; x is replicated).

Formulation per core (transposed): for each head a and hid-tile m,
  hT[m] = gelu(W1[a,:,m128].T @ x.T + b1) as [128 hid, 512 batch] tiles,
computed on the PE with float32r operands (full bf16-rate at N=512, ~1e-4
rel err vs the fp32 reference -- measured 6.5e-5 end to end), gelu+bias
fused on ScalarE out of PSUM, then the per-head dot product accumulates
over m as M=1 matmuls into a second PSUM bank (emitted one stage-1 group
late so the in-order PE queue never waits on ACT). x.T is resident in SBUF
as batch halves; W1 streams from HBM (packed host-side for contiguous
per-(a,m) 1 MiB DMAs). Sigmoid+b2 run once at the end (one extra
activation-table load).
"""
import os
import sys
from contextlib import ExitStack

import numpy as np

for _p in ("/root/.axon_site/_ro/trn_rl_repo", "/opt/trn_rl_repo"):
    if os.path.isdir(_p) and _p not in sys.path:
        sys.path.append(_p)

import jax  # noqa: E402
from jax.sharding import Mesh, PartitionSpec, NamedSharding  # noqa: E402
from jax.experimental.shard_map import shard_map  # noqa: E402

import concourse.bacc as bacc  # noqa: E402
import concourse.tile as tile  # noqa: E402
from concourse import mybir, bass2jax  # noqa: E402

F32 = mybir.dt.float32
F32R = mybir.dt.float32r
AF = mybir.ActivationFunctionType

# problem shape (hardcoded; see module docstring)
B, D, A, H = 4096, 2048, 40, 1024
NCORES = 8
APC = A // NCORES        # 5 heads per core
KT = D // 128            # 16 contraction tiles
MT = H // 128            # 8 hid tiles
NQ = 4                   # batch blocks (4 => double-buffered quarters; W1 streamed 4x,
                         # ~44% DMA duty, but batch-block reloads fully overlap compute)
QTR = B // NQ            # resident batch block
NCH = QTR // 512         # 512-wide chunks per block


def build_program(repeat: int = 0, nq: int = NQ):
    qtr = B // nq
    nch = qtr // 512
    xbufs = 2 if nq >= 4 else 1
    nc = bacc.Bacc("TRN2", target_bir_lowering=False, debug=False)
    xT_d = nc.dram_tensor("xT", [D, B], F32R, kind="ExternalInput").ap()
    w1_d = nc.dram_tensor("w1p", [APC, MT, 128, KT * 128], F32R, kind="ExternalInput").ap()
    b1_d = nc.dram_tensor("b1p", [APC, 128, MT], F32, kind="ExternalInput").ap()
    w2_d = nc.dram_tensor("w2p", [APC, 128, MT], F32R, kind="ExternalInput").ap()
    b2_d = nc.dram_tensor("b2p", [APC, 1], F32, kind="ExternalInput").ap()
    y_d = nc.dram_tensor("y", [APC, B], F32, kind="ExternalOutput").ap()

    with tile.TileContext(nc) as tc, ExitStack() as ctx:
        const = ctx.enter_context(tc.tile_pool(name="const", bufs=1))
        xp = ctx.enter_context(tc.tile_pool(name="xp", bufs=xbufs))
        wp = ctx.enter_context(tc.tile_pool(name="wp", bufs=2))
        sp = ctx.enter_context(tc.tile_pool(name="sp", bufs=3))
        hp = ctx.enter_context(tc.tile_pool(name="hp", bufs=5))
        lg = ctx.enter_context(tc.tile_pool(name="lg", bufs=1))
        ps1 = ctx.enter_context(tc.tile_pool(name="ps1", bufs=4, space="PSUM"))
        ps2 = ctx.enter_context(tc.tile_pool(name="ps2", bufs=4, space="PSUM"))

        def body():
            b1t = const.tile([128, APC * MT], F32, tag="b1t")
            w2t = const.tile([128, APC * MT], F32R, tag="w2t")
            b2t = const.tile([APC, 1], F32, tag="b2t")
            for a in range(APC):
                nc.sync.dma_start(b1t[:, a * MT:(a + 1) * MT], b1_d[a])
                nc.sync.dma_start(w2t[:, a * MT:(a + 1) * MT], w2_d[a])
            nc.sync.dma_start(b2t[:], b2_d[:])
            logits = lg.tile([APC, B], F32, tag="logits")
            for hf in range(nq):
                xq = []
                for k in range(KT):
                    t = xp.tile([128, qtr], F32R, tag=f"xq{k}")
                    nc.sync.dma_start(t[:], xT_d[k * 128:(k + 1) * 128,
                                                 hf * qtr:(hf + 1) * qtr])
                    xq.append(t)
                for a in range(APC):
                    psy = [None] * nch
                    # stage-2 matmuls are emitted one stage-1 group late so
                    # the in-order PE queue never waits on the gelu (ACT)
                    # that produces their rhs.
                    pending = []
                    for m in range(MT):
                        w = wp.tile([128, KT * 128], F32R, tag="w")
                        nc.sync.dma_start(w[:], w1_d[a, m])
                        # Boundary iterations run k-outermost so xq k-tiles
                        # are first-needed / last-read staggered by k: the
                        # next half's 16 MiB xq reload then overlaps compute
                        # instead of stalling the PE at the half boundary.
                        kouter = (m == 0 and a == 0) or \
                                 (m == MT - 1 and a == APC - 1)
                        if kouter:
                            pts = []
                            for n in range(nch):
                                pt_n = ps1.tile([128, 512], F32, tag="ps1",
                                                name=f"pt{n}")
                                pts.append(pt_n)
                            for k in range(KT):
                                for n in range(nch):
                                    nc.tensor.matmul(
                                        pts[n][:],
                                        w[:, k * 128:(k + 1) * 128],
                                        xq[k][:, n * 512:(n + 1) * 512],
                                        start=(k == 0), stop=(k == KT - 1))
                            while pending:
                                pending.pop(0)()
                        def tail(n, pt, m=m):
                            ht = hp.tile([128, 512], F32R, tag="ht",
                                         name="ht")
                            nc.scalar.activation(
                                ht[:], pt[:], AF.Gelu,
                                bias=b1t[:, a * MT + m:a * MT + m + 1])
                            if m == 0:
                                psy_t = ps2.tile([1, 512], F32, tag="psy",
                                                 name="psy_t")
                                psy[n] = psy_t

                            def emit_stage2(m=m, n=n, ht=ht):
                                nc.tensor.matmul(
                                    psy[n][:],
                                    w2t[:, a * MT + m:a * MT + m + 1],
                                    ht[:],
                                    start=(m == 0), stop=(m == MT - 1),
                                    skip_group_check=True)
                            pending.append(emit_stage2)

                        if kouter:
                            for n in range(nch):
                                tail(n, pts[n])
                        else:
                            for n in range(nch):
                                pt = ps1.tile([128, 512], F32, tag="ps1")
                                for k in range(KT):
                                    nc.tensor.matmul(
                                        pt[:],
                                        w[:, k * 128:(k + 1) * 128],
                                        xq[k][:, n * 512:(n + 1) * 512],
                                        start=(k == 0), stop=(k == KT - 1))
                                if pending:
                                    pending.pop(0)()
                                tail(n, pt)
                    while pending:
                        pending.pop(0)()
                    for n in range(nch):
                        stg = sp.tile([1, 512], F32, tag="stg")
                        nc.vector.tensor_copy(stg[:], psy[n][:])
                        nc.sync.dma_start(
                            logits[a:a + 1,
                                   hf * qtr + n * 512:hf * qtr + (n + 1) * 512],
                            stg[:])
            yt = lg.tile([APC, B], F32, tag="yt")
            nc.scalar.activation(yt[:], logits[:], AF.Sigmoid, bias=b2t[:])
            nc.sync.dma_start(y_d[:], yt[:])

        if repeat and repeat > 1:
            with tc.For_i(0, repeat, 1):
                body()
        else:
            body()
    nc.compile()
    return nc


class _Runner:
    """jit-once PJRT runner for a prebuilt Bass program (8-core SPMD)."""

    def __init__(self, nc, n_cores):
        bass2jax.install_neuronx_cc_hook()
        self.nc = nc
        self.n_cores = n_cores
        in_names, out_names, out_avals, zero_outs = [], [], [], []
        for alloc in nc.m.functions[0].allocations:
            if not isinstance(alloc, mybir.MemoryLocationSet):
                continue
            name = alloc.memorylocations[0].name
            if alloc.kind == "ExternalInput":
                in_names.append(name)
            elif alloc.kind == "ExternalOutput":
                shape = tuple(alloc.tensor_shape)
                dtype = mybir.dt.np(alloc.dtype)
                out_names.append(name)
                out_avals.append(jax.core.ShapedArray(shape, dtype))
                zero_outs.append(np.zeros(shape, dtype))
        partition_name = (nc.partition_id_tensor.name
                          if nc.partition_id_tensor else None)
        if partition_name is not None and partition_name in in_names:
            in_names.remove(partition_name)
        self.in_names = in_names
        self.out_names = out_names
        self.zero_outs = zero_outs
        n_params = len(in_names)
        n_outs = len(out_avals)
        all_in_names = list(in_names) + list(out_names)
        if partition_name is not None:
            all_in_names.append(partition_name)
        donate = tuple(range(n_params, n_params + n_outs))

        def _body(*args):
            operands = list(args)
            if partition_name is not None:
                operands.append(bass2jax.partition_id_tensor())
            outs = bass2jax._bass_exec_p.bind(
                *operands,
                out_avals=tuple(out_avals),
                in_names=tuple(all_in_names),
                out_names=tuple(out_names),
                lowering_input_output_aliases=(),
                sim_require_finite=True,
                sim_require_nnan=True,
                nc=nc,
            )
            return tuple(outs)

        devices = jax.devices()[:n_cores]
        assert len(devices) == n_cores, f"need {n_cores} neuron cores"
        self.mesh = Mesh(np.asarray(devices), ("core",))
        in_specs = (PartitionSpec("core"),) * (n_params + n_outs)
        out_specs = (PartitionSpec("core"),) * n_outs
        self.fn = jax.jit(
            shard_map(_body, mesh=self.mesh, in_specs=in_specs,
                      out_specs=out_specs, check_rep=False),
            donate_argnums=donate, keep_unused=True,
        )
        self._dev_inputs = None

    def put_inputs(self, in_maps):
        sharding = NamedSharding(self.mesh, PartitionSpec("core"))
        self._dev_inputs = [
            jax.device_put(
                np.concatenate([np.asarray(m[name]) for m in in_maps], axis=0),
                sharding)
            for name in self.in_names
        ]

    def run(self):
        sharding = NamedSharding(self.mesh, PartitionSpec("core"))
        zouts = [jax.device_put(np.concatenate([z] * self.n_cores, axis=0),
                                sharding) for z in self.zero_outs]
        outs = self.fn(*self._dev_inputs, *zouts)
        jax.block_until_ready(outs)
        return outs

    def run_np(self):
        outs = self.run()
        res = []
        for c in range(self.n_cores):
            d = {}
            for i, name in enumerate(self.out_names):
                full = np.asarray(outs[i])
                per = full.shape[0] // self.n_cores
                d[name] = full[c * per:(c + 1) * per]
            res.append(d)
        return res


_CACHE = {}


def _get_runner(repeat=0):
    if repeat not in _CACHE:
        _CACHE[repeat] = _Runner(build_program(repeat), NCORES)
    return _CACHE[repeat]


def make_in_maps(x, W1, b1, W2, b2):
    x = np.asarray(x, dtype=np.float32)
    W1 = np.asarray(W1, dtype=np.float32)
    b1 = np.asarray(b1, dtype=np.float32)
    W2 = np.asarray(W2, dtype=np.float32)
    b2 = np.asarray(b2, dtype=np.float32)
    xT = np.ascontiguousarray(x.T)
    # W1p[a, m, p, k*128+c] = W1[a, k*128+p, m*128+c]  (per-(a,m) contiguous
    # 1 MiB block whose partition rows are 8 KiB contiguous runs)
    W1p = np.ascontiguousarray(
        W1.reshape(A, KT, 128, MT, 128).transpose(0, 3, 2, 1, 4)
        .reshape(A, MT, 128, KT * 128))
    b1p = np.ascontiguousarray(b1.reshape(A, MT, 128).transpose(0, 2, 1))
    W2p = np.ascontiguousarray(W2.reshape(A, MT, 128).transpose(0, 2, 1))
    b2p = np.ascontiguousarray(b2.reshape(A, 1))
    in_maps = []
    for c in range(NCORES):
        s = slice(c * APC, (c + 1) * APC)
        in_maps.append({"xT": xT, "w1p": W1p[s], "b1p": b1p[s],
                        "w2p": W2p[s], "b2p": b2p[s]})
    return in_maps


def kernel(x, W1, b1, W2, b2):
    in_maps = make_in_maps(x, W1, b1, W2, b2)
    r = _get_runner(0)
    r.put_inputs(in_maps)
    outs = r.run_np()
    y = np.concatenate([outs[c]["y"] for c in range(NCORES)], axis=0)
    return np.ascontiguousarray(y.T).astype(np.float32)



# revision 5
# speedup vs baseline: 2.3740x; 2.3740x over previous
"""TRN2 Bass kernel for nn_AttributeClassifierHeaders (dense per-head MLP).

Computes y[b, a] = sigmoid(gelu(x @ W1[a] + b1[a]) . W2[a] + b2[a]) for 40
heads, sharded 5 heads per NeuronCore across 8 cores (head-parallel: each
head's weights are independent; x is replicated).

Stage-1 (the 2048x1024 per-head GEMM, 97% of FLOPs) runs on the PE in
fp8-e4m3 with perf_mode=DoubleRow: two fp8 weights per PE cell contract
K=256 per matmul at 0.5 cycles/row (2x the fp32r/bf16 rate), and fp8
halves W1 traffic so the whole problem needs a single W1 pass with x
resident in SBUF (no batch-block reload loop at all). x is quantized
host-side to e4m3*2^5 and W1 to e4m3*2^13; the 2^-18 descale rides the
gelu's scale operand on ScalarE (out = gelu(psum*2^-18 + b1)). Measured
end-to-end rel err vs the fp32 reference: ~1.3e-2 (tolerance 2e-2);
stage-2 (the per-head dot) runs in bf16 (negligible error), emitted one
gelu-group late so the in-order PE queue never waits on ACT. Stage-2 must
NOT be f32r: the PE pulls LDWEIGHTS ahead of in-flight matmuls, and an
fp8-DR LDWEIGHTS hoisted into the middle of an f32r matmul's internal
two-pass 4-byte weight load corrupts it (hw-bisected: stage-1 h dumps
were bit-exact while f32r stage-2 logits were wrong).

Layouts (host-packed):
  x8[kb, p, i, b]    = e4m3(x[b, kb*256 + i*128 + p] * 2^5)   moving
  w18[a, m, p, kb, i, c] = e4m3(W1[a, kb*256+i*128+p, m*128+c] * 2^13)
so each DoubleRow matmul takes stationary [128, 2, 128] (i-stride 128 B)
and moving [128, 2, 512] (i-stride B bytes), matching the hw's
[Ki, Ko=2, dim] access-pattern requirement (step % 16 == 0).
"""
import os
import sys
from contextlib import ExitStack

import numpy as np
import ml_dtypes

for _p in ("/root/.axon_site/_ro/trn_rl_repo", "/opt/trn_rl_repo"):
    if os.path.isdir(_p) and _p not in sys.path:
        sys.path.append(_p)

import jax  # noqa: E402
from jax.sharding import Mesh, PartitionSpec, NamedSharding  # noqa: E402
from jax.experimental.shard_map import shard_map  # noqa: E402

import concourse.bacc as bacc  # noqa: E402
import concourse.tile as tile  # noqa: E402
from concourse import mybir, bass2jax  # noqa: E402

F32 = mybir.dt.float32
F32R = mybir.dt.float32r
BF16 = mybir.dt.bfloat16
F8 = mybir.dt.float8e4
AF = mybir.ActivationFunctionType
DR = mybir.MatmulPerfMode.DoubleRow

# problem shape (hardcoded; see module docstring)
B, D, A, H = 4096, 2048, 40, 1024
NCORES = 8
APC = A // NCORES        # 5 heads per core
KB = D // 256            # 8 contraction blocks of 256 (DoubleRow)
MT = H // 128            # 8 hid tiles
XS = 5                   # x quant scale exponent  (x * 2^5)
WS = 13                  # W1 quant scale exponent (W1 * 2^13)
DESCALE = 2.0 ** (-(XS + WS))


def build_program(repeat: int = 0, apc: int = APC, b: int = B,
                  gelu_af=AF.Gelu):
    nch = b // 512
    nc = bacc.Bacc("TRN2", target_bir_lowering=False, debug=False)
    x8_d = nc.dram_tensor("x8", [KB, 128, 2 * b], F8, kind="ExternalInput").ap()
    w1_d = nc.dram_tensor("w1p", [apc, MT, 128, KB * 256], F8,
                          kind="ExternalInput").ap()
    b1_d = nc.dram_tensor("b1p", [apc, 128, MT], F32, kind="ExternalInput").ap()
    w2_d = nc.dram_tensor("w2p", [apc, 128, MT], BF16, kind="ExternalInput").ap()
    b2_d = nc.dram_tensor("b2p", [apc, 1], F32, kind="ExternalInput").ap()
    y_d = nc.dram_tensor("y", [apc, b], F32, kind="ExternalOutput").ap()

    with tile.TileContext(nc) as tc, ExitStack() as ctx:
        const = ctx.enter_context(tc.tile_pool(name="const", bufs=1))
        xp = ctx.enter_context(tc.tile_pool(name="xp", bufs=1))
        wp = ctx.enter_context(tc.tile_pool(name="wp", bufs=2))
        sp = ctx.enter_context(tc.tile_pool(name="sp", bufs=3))
        hp = ctx.enter_context(tc.tile_pool(name="hp", bufs=5))
        lg = ctx.enter_context(tc.tile_pool(name="lg", bufs=1))
        ps1 = ctx.enter_context(tc.tile_pool(name="ps1", bufs=4, space="PSUM"))
        ps2 = ctx.enter_context(tc.tile_pool(name="ps2", bufs=4, space="PSUM"))

        def dma_head(a):
            tiles = []
            for m in range(MT):
                t = wp.tile([128, KB, 2, 128], F8, tag=f"w{m}")
                nc.sync.dma_start(t[:], w1_d[a, m])
                tiles.append(t)
            return tiles

        def body():
            b1t = const.tile([128, apc * MT], F32, tag="b1t")
            w2t = const.tile([128, apc * MT], BF16, tag="w2t")
            b2t = const.tile([apc, 1], F32, tag="b2t")
            for a in range(apc):
                nc.sync.dma_start(b1t[:, a * MT:(a + 1) * MT], b1_d[a])
                nc.sync.dma_start(w2t[:, a * MT:(a + 1) * MT], w2_d[a])
            nc.sync.dma_start(b2t[:], b2_d[:])
            xq = []
            for kb in range(KB):
                t = xp.tile([128, 2, b], F8, tag=f"xq{kb}")
                nc.sync.dma_start(t[:], x8_d[kb])
                xq.append(t)
            logits = lg.tile([apc, b], F32, tag="logits")
            wcur = dma_head(0)
            for a in range(apc):
                wnxt = dma_head(a + 1) if a + 1 < apc else None
                for n in range(nch):
                    psy = ps2.tile([1, 512], F32, tag="psy")
                    # stage-2 matmuls are emitted one gelu-group late so the
                    # in-order PE queue never waits on the ACT that produces
                    # their rhs.
                    pending = []
                    for m in range(MT):
                        pt = ps1.tile([128, 512], F32, tag="ps1")
                        for kb in range(KB):
                            nc.tensor.matmul(
                                pt[:],
                                wcur[m][:, kb],
                                xq[kb][:, :, n * 512:(n + 1) * 512],
                                start=(kb == 0), stop=(kb == KB - 1),
                                perf_mode=DR)
                        if pending:
                            pending.pop(0)()
                        ht = hp.tile([128, 512], BF16, tag="ht", name="ht")
                        nc.scalar.activation(
                            ht[:], pt[:], gelu_af,
                            bias=b1t[:, a * MT + m:a * MT + m + 1],
                            scale=DESCALE)

                        def emit_stage2(m=m, ht=ht):
                            nc.tensor.matmul(
                                psy[:],
                                w2t[:, a * MT + m:a * MT + m + 1],
                                ht[:],
                                start=(m == 0), stop=(m == MT - 1),
                                skip_group_check=True)
                        pending.append(emit_stage2)
                    while pending:
                        pending.pop(0)()
                    stg = sp.tile([1, 512], F32, tag="stg")
                    nc.vector.tensor_copy(stg[:], psy[:])
                    nc.sync.dma_start(
                        logits[a:a + 1, n * 512:(n + 1) * 512], stg[:])
                wcur = wnxt
            yt = lg.tile([apc, b], F32, tag="yt")
            nc.scalar.activation(yt[:], logits[:], AF.Sigmoid, bias=b2t[:])
            nc.sync.dma_start(y_d[:], yt[:])

        if repeat and repeat > 1:
            with tc.For_i(0, repeat, 1):
                body()
        else:
            body()
    nc.compile()
    return nc


class _Runner:
    """jit-once PJRT runner for a prebuilt Bass program (8-core SPMD)."""

    def __init__(self, nc, n_cores):
        bass2jax.install_neuronx_cc_hook()
        self.nc = nc
        self.n_cores = n_cores
        in_names, out_names, out_avals, zero_outs = [], [], [], []
        for alloc in nc.m.functions[0].allocations:
            if not isinstance(alloc, mybir.MemoryLocationSet):
                continue
            name = alloc.memorylocations[0].name
            if alloc.kind == "ExternalInput":
                in_names.append(name)
            elif alloc.kind == "ExternalOutput":
                shape = tuple(alloc.tensor_shape)
                dtype = mybir.dt.np(alloc.dtype)
                out_names.append(name)
                out_avals.append(jax.core.ShapedArray(shape, dtype))
                zero_outs.append(np.zeros(shape, dtype))
        partition_name = (nc.partition_id_tensor.name
                          if nc.partition_id_tensor else None)
        if partition_name is not None and partition_name in in_names:
            in_names.remove(partition_name)
        self.in_names = in_names
        self.out_names = out_names
        self.zero_outs = zero_outs
        n_params = len(in_names)
        n_outs = len(out_avals)
        all_in_names = list(in_names) + list(out_names)
        if partition_name is not None:
            all_in_names.append(partition_name)
        donate = tuple(range(n_params, n_params + n_outs))

        def _body(*args):
            operands = list(args)
            if partition_name is not None:
                operands.append(bass2jax.partition_id_tensor())
            outs = bass2jax._bass_exec_p.bind(
                *operands,
                out_avals=tuple(out_avals),
                in_names=tuple(all_in_names),
                out_names=tuple(out_names),
                lowering_input_output_aliases=(),
                sim_require_finite=True,
                sim_require_nnan=True,
                nc=nc,
            )
            return tuple(outs)

        devices = jax.devices()[:n_cores]
        assert len(devices) == n_cores, f"need {n_cores} neuron cores"
        self.mesh = Mesh(np.asarray(devices), ("core",))
        in_specs = (PartitionSpec("core"),) * (n_params + n_outs)
        out_specs = (PartitionSpec("core"),) * n_outs
        self.fn = jax.jit(
            shard_map(_body, mesh=self.mesh, in_specs=in_specs,
                      out_specs=out_specs, check_rep=False),
            donate_argnums=donate, keep_unused=True,
        )
        self._dev_inputs = None

    def put_inputs(self, in_maps):
        sharding = NamedSharding(self.mesh, PartitionSpec("core"))
        self._dev_inputs = [
            jax.device_put(
                np.concatenate([np.asarray(m[name]) for m in in_maps], axis=0),
                sharding)
            for name in self.in_names
        ]

    def run(self):
        sharding = NamedSharding(self.mesh, PartitionSpec("core"))
        zouts = [jax.device_put(np.concatenate([z] * self.n_cores, axis=0),
                                sharding) for z in self.zero_outs]
        outs = self.fn(*self._dev_inputs, *zouts)
        jax.block_until_ready(outs)
        return outs

    def run_np(self):
        outs = self.run()
        res = []
        for c in range(self.n_cores):
            d = {}
            for i, name in enumerate(self.out_names):
                full = np.asarray(outs[i])
                per = full.shape[0] // self.n_cores
                d[name] = full[c * per:(c + 1) * per]
            res.append(d)
        return res


_CACHE = {}


def _get_runner(repeat=0):
    if repeat not in _CACHE:
        _CACHE[repeat] = _Runner(build_program(repeat), NCORES)
    return _CACHE[repeat]


def _f8(v, sexp):
    return np.asarray(v * (2.0 ** sexp), dtype=ml_dtypes.float8_e4m3)


def pack_x(x):
    # x8[kb, p, i*B + col] = e4m3(x[col, kb*256 + i*128 + p] * 2^XS)
    b = x.shape[0]
    x8 = _f8(np.asarray(x, dtype=np.float32), XS)        # [B, D]
    x8 = np.ascontiguousarray(
        x8.T.reshape(KB, 2, 128, b).transpose(0, 2, 1, 3).reshape(
            KB, 128, 2 * b))
    return x8


def pack_w1(W1):
    # w18[a, m, p, kb*256 + i*128 + c] = e4m3(W1[a, kb*256+i*128+p, m*128+c]
    #                                          * 2^WS)
    a = W1.shape[0]
    w8 = _f8(np.asarray(W1, dtype=np.float32), WS)       # [A, D, H]
    w8 = np.ascontiguousarray(
        w8.reshape(a, KB, 2, 128, MT, 128).transpose(0, 4, 3, 1, 2, 5)
        .reshape(a, MT, 128, KB * 256))
    return w8


def make_in_maps(x, W1, b1, W2, b2):
    b1 = np.asarray(b1, dtype=np.float32)
    W2 = np.asarray(W2, dtype=np.float32)
    b2 = np.asarray(b2, dtype=np.float32)
    x8 = pack_x(x)
    W1p = pack_w1(W1)
    b1p = np.ascontiguousarray(b1.reshape(A, MT, 128).transpose(0, 2, 1))
    W2p = np.ascontiguousarray(W2.reshape(A, MT, 128).transpose(0, 2, 1)
                           .astype(ml_dtypes.bfloat16))
    b2p = np.ascontiguousarray(b2.reshape(A, 1))
    in_maps = []
    for c in range(NCORES):
        s = slice(c * APC, (c + 1) * APC)
        in_maps.append({"x8": x8, "w1p": W1p[s], "b1p": b1p[s],
                        "w2p": W2p[s], "b2p": b2p[s]})
    return in_maps


def kernel(x, W1, b1, W2, b2):
    in_maps = make_in_maps(x, W1, b1, W2, b2)
    r = _get_runner(0)
    r.put_inputs(in_maps)
    outs = r.run_np()
    y = np.concatenate([outs[c]["y"] for c in range(NCORES)], axis=0)
    return np.ascontiguousarray(y.T).astype(np.float32)
